# revision 37
# baseline (speedup 1.0000x reference)
"""AJ-RNN (2-layer LSTM with missing-value imputation) on 8 TRN2 NeuronCores.

Sharding: data-parallel over batch (B=256 -> 32 rows/core), weights replicated,
no collectives. Per core, the T=256 sequential scan runs fully SBUF-resident.

Key hardware facts this design is built around (probed on this toolchain):
  - fp32r matmuls run at 1 cycle/row for N>=256 (4x faster than fp32) but
    REQUIRE dst partition 0 -> no column-tiling; z stays flat [32, 4H].
  - fp32r operands must be produced by an instruction that rounds to fp32r
    (DVE/ACT op with fp32r out dtype); raw DMA data doesn't qualify.
  - tensor_tensor INPUTS must share a start partition; outputs and ACT are
    free. With flat z everything lives at base partition 0 anyway.
  - bf16 anywhere in the recurrence blows the 2e-2 error budget; fp32r
    matmuls + fp32 elementwise measure ~3e-5 end-to-end.

Per-step layout (B_l = 32 batch rows/core):
  - Gate columns host-permuted to (i, f, o, g) so one sigmoid covers
    z[:, 0:1536] and one tanh covers z[:, 1536:2048].
  - b0 folded into the x-projection as a K=33 matmul (cur_aug has a ones row).
  - States kept transposed (hT [128, 4*32]) feeding matmuls as lhsT;
    h -> hT via 4 PE transposes + one fp32r-rounding copy.
  - Imputation: cur^T = xmb^T[t] + mask^T[t] * predraw^T with
    xmb = where(miss,0,x) + miss*bproj precomputed on host;
    predraw^T = Wproj^T @ h1^T computed on-chip (= the `prediction` output).
"""

import numpy as np

import concourse.bass as bass
import concourse.mybir as mybir
from concourse import bacc, masks, tile
from concourse.bass_utils import run_bass_kernel_spmd

F32 = mybir.dt.float32
F32R = mybir.dt.float32r
AF = mybir.ActivationFunctionType
OP = mybir.AluOpType

B, T, D, H, C = 256, 256, 32, 512, 10
NCORES = 8
BL = B // NCORES  # 32
MISSING = 128.0
CHUNK = 16  # io staging chunk, in steps
KC = H // 128  # 4 K-chunks

# host-side gate-column permutation: (i, f, o, g)
GATE_PERM = np.concatenate([
    np.arange(0, H), np.arange(H, 2 * H),
    np.arange(3 * H, 4 * H), np.arange(2 * H, 3 * H)])
# per-gate emission order (f, g, i, o): f's bank finishes first so its
# sigmoid (and the f*c multiply) start while other gates still stream
GORDER = (1, 3, 0, 2)


def r(ap):
    return ap.bitcast(F32R)


def build(t_steps=T, zero_b1=True):
    nc = bacc.Bacc("TRN2", target_bir_lowering=False, debug=False)

    NP = t_steps - 1

    x0T_d = nc.dram_tensor("x0T", [D, BL], F32, kind="ExternalInput")
    xmb_d = nc.dram_tensor("xmbT", [D, NP * BL], F32, kind="ExternalInput")
    msk_d = nc.dram_tensor("maskT", [D, NP * BL], F32, kind="ExternalInput")
    wx0b_d = nc.dram_tensor("Wx0b", [D + 1, 4 * H], F32, kind="ExternalInput")
    u0_d = nc.dram_tensor("U0", [H, 4 * H], F32, kind="ExternalInput")
    wx1_d = nc.dram_tensor("Wx1", [H, 4 * H], F32, kind="ExternalInput")
    u1_d = nc.dram_tensor("U1", [H, 4 * H], F32, kind="ExternalInput")
    b1_d = nc.dram_tensor("b1", [1, 4 * H], F32, kind="ExternalInput")
    wproj_d = nc.dram_tensor("Wproj", [H, D], F32, kind="ExternalInput")
    bprojv_d = nc.dram_tensor("bprojv", [D, 1], F32, kind="ExternalInput")
    wcls_d = nc.dram_tensor("Wcls", [H, C], F32, kind="ExternalInput")
    bcls_d = nc.dram_tensor("bcls", [1, C], F32, kind="ExternalInput")

    pred_d = nc.dram_tensor("pred_out", [D, NP * BL], F32, kind="ExternalOutput")
    last_d = nc.dram_tensor("last_out", [BL, H], F32, kind="ExternalOutput")
    logit_d = nc.dram_tensor("logits_out", [BL, C], F32, kind="ExternalOutput")

    with tile.TileContext(nc) as tc:
        with (
            tc.tile_pool(name="weights", bufs=1) as wpool,
            tc.tile_pool(name="io", bufs=3) as iopool,
            tc.tile_pool(name="state_h", bufs=4) as hpool,
            tc.tile_pool(name="state_c", bufs=3) as cpool,
            tc.tile_pool(name="gates", bufs=2) as gpool,
            tc.tile_pool(name="zps", bufs=7, space="PSUM") as zpool,
            tc.tile_pool(name="prps", bufs=1, space="PSUM") as prpool,
        ):
            wx0b_s = wpool.tile([D + 1, 4 * H], F32, tag="wx0b")
            u0_s = wpool.tile([128, KC * 4 * H], F32, tag="u0")
            wx1_s = wpool.tile([128, KC * 4 * H], F32, tag="wx1")
            u1_s = wpool.tile([128, KC * 4 * H], F32, tag="u1")
            wproj_s = wpool.tile([128, KC * D], F32, tag="wproj")
            wcls_s = wpool.tile([128, KC * C], F32, tag="wcls")
            b1_s = wpool.tile([1, 4 * H], F32, tag="b1")
            bprojv_s = wpool.tile([D, 1], F32, tag="bprojv")
            bcls_s = wpool.tile([1, C], F32, tag="bcls")
            ones_s = wpool.tile([1, BL], F32, tag="ones")
            x0T_s = wpool.tile([D + 1, BL], F32, tag="x0T")

            # fp32r operands must come from a rounding instruction:
            # DMA -> staging -> rounding copy -> resident tile.
            with tc.tile_pool(name="wstage", bufs=2) as wstage:
                def load_rounded(dst_slice, src_ap, shape):
                    if shape[1] > 2 * H:
                        half = shape[1] // 2
                        load_rounded(dst_slice[:, 0:half], src_ap[:, 0:half],
                                     (shape[0], half))
                        load_rounded(dst_slice[:, half:], src_ap[:, half:],
                                     (shape[0], shape[1] - half))
                        return
                    st = wstage.tile([128, 2 * H], F32, tag="wst")
                    nc.sync.dma_start(st[0:shape[0], 0:shape[1]], src_ap)
                    nc.vector.tensor_copy(
                        dst_slice.bitcast(F32R), st[0:shape[0], 0:shape[1]])

                load_rounded(wx0b_s[:], wx0b_d[:], (D + 1, 4 * H))
                for k in range(KC):
                    sl = slice(128 * k, 128 * (k + 1))
                    load_rounded(u0_s[:, 4 * H * k:4 * H * (k + 1)], u0_d[sl, :], (128, 4 * H))
                    load_rounded(wx1_s[:, 4 * H * k:4 * H * (k + 1)], wx1_d[sl, :], (128, 4 * H))
                    load_rounded(u1_s[:, 4 * H * k:4 * H * (k + 1)], u1_d[sl, :], (128, 4 * H))
                    load_rounded(wproj_s[:, D * k:D * (k + 1)], wproj_d[sl, :], (128, D))
                    load_rounded(wcls_s[:, C * k:C * (k + 1)], wcls_d[sl, :], (128, C))
                load_rounded(b1_s[:], b1_d[:], (1, 4 * H))
                load_rounded(bcls_s[:], bcls_d[:], (1, C))
                load_rounded(x0T_s[0:D, :], x0T_d[:], (D, BL))
            nc.sync.dma_start(bprojv_s[:], bprojv_d[:])
            nc.vector.memset(ones_s[:], 1.0)
            nc.vector.memset(x0T_s[D:D + 1, :], 1.0)

            def hT_chunks(hT):
                return [hT[:, 32 * k:32 * (k + 1)] for k in range(KC)]

            def gates(zb, c_prev, first):
                """zb: 4 PSUM bank tiles in gate order (i, f, o, g).
                ACT/DVE ops emitted in dependency-earliest order (f, g, i, o)
                so they start as soon as each gate's bank finishes."""
                sgf = gpool.tile([32, H], F32, tag="sgf")
                tg = gpool.tile([32, H], F32, tag="tg")
                sgi = gpool.tile([32, H], F32, tag="sgi")
                sgo = gpool.tile([32, H], F32, tag="sgo")
                nc.scalar.activation(sgf[:], zb[1][:], AF.Sigmoid)
                nc.scalar.activation(tg[:], zb[3][:], AF.Tanh)
                nc.scalar.activation(sgi[:], zb[0][:], AF.Sigmoid)
                nc.scalar.activation(sgo[:], zb[2][:], AF.Sigmoid)
                c_new = cpool.tile([32, H], F32, tag="c")
                if first:
                    nc.vector.tensor_tensor(c_new[:], sgi[:], tg[:], op=OP.mult)
                else:
                    fc = gpool.tile([32, H], F32, tag="fc")
                    nc.vector.tensor_tensor(fc[:], sgf[:], c_prev[:], op=OP.mult)
                    m2 = gpool.tile([32, H], F32, tag="m2")
                    nc.vector.tensor_tensor(m2[:], sgi[:], tg[:], op=OP.mult)
                    nc.vector.tensor_tensor(c_new[:], fc[:], m2[:], op=OP.add)
                tc_t = gpool.tile([32, H], F32, tag="tc")
                nc.scalar.activation(tc_t[:], c_new[:], AF.Tanh)
                h = gpool.tile([32, H], F32, tag="h")
                nc.vector.tensor_tensor(h[:], sgo[:], tc_t[:], op=OP.mult)
                return h, c_new

            def transpose_h(h):
                """h [32,512] -> hT [128, 4*32] without touching the PE:
                DVE 32x32 block-transpose, then 16 [32,32] gather copies
                (which also do the fp32r rounding), k-major so the first
                K-chunk of hT is consumable early."""
                tr = gpool.tile([32, H], F32, tag="htr")
                nc.vector.transpose(tr[:], h[:])
                hT = hpool.tile([128, KC * 32], F32, tag="hT")
                for k in range(KC):
                    for c in range(4):
                        dst = r(hT)[32 * c:32 * (c + 1), 32 * k:32 * (k + 1)]
                        src = tr[:, 32 * (4 * k + c):32 * (4 * k + c) + 32]
                        if k < 2:
                            nc.vector.tensor_copy(dst, src)
                        else:
                            nc.scalar.copy(dst, src)
                return hT

            h0T = h1T = c0 = c1 = None
            xmb_s = msk_s = po_s = None

            for t in range(t_steps):
                j = t - 1  # imputation/prediction row index
                cj = j % CHUNK
                if t >= 1 and cj == 0:
                    n = min(CHUNK, NP - j) * BL
                    xmb_s = iopool.tile([D, CHUNK * BL], F32, tag="xmb")
                    msk_s = iopool.tile([D, CHUNK * BL], F32, tag="msk")
                    po_s = iopool.tile([D, CHUNK * BL], F32, tag="po")
                    nc.sync.dma_start(xmb_s[:, 0:n], xmb_d[:, j * BL:j * BL + n])
                    nc.sync.dma_start(msk_s[:, 0:n], msk_d[:, j * BL:j * BL + n])

                # ---- layer-0 recurrent matmuls first: they fill the PE
                # while gates1(t-1) is still on ACT/DVE ----
                z0b = [zpool.tile([32, H], F32, tag="zg", name=f"z0g{g}")
                       for g in range(4)]
                if t > 0:
                    for g in GORDER:
                        cs = slice(H * g, H * (g + 1))
                        for k in range(KC):
                            nc.tensor.matmul(
                                z0b[g][:], r(h0T[:, 32 * k:32 * (k + 1)]),
                                r(u0_s[:, 4 * H * k + H * g:4 * H * k + H * (g + 1)]),
                                start=(k == 0), stop=False)

                if t == 0:
                    cur = x0T_s
                else:
                    # predraw^T = Wproj^T @ h1T  [D, BL]
                    pp = prpool.tile([D, BL], F32, tag="prps")
                    for k in range(KC):
                        nc.tensor.matmul(
                            pp[:], r(wproj_s[:, D * k:D * (k + 1)]),
                            r(h1T[:, 32 * k:32 * (k + 1)]),
                            start=(k == 0), stop=(k == KC - 1),
                        )
                    nc.vector.tensor_scalar_add(
                        po_s[:, cj * BL:(cj + 1) * BL], pp[:], bprojv_s[:])
                    # cur^T = xmb^T + mask^T * predraw^T
                    mp = iopool.tile([D, BL], F32, tag="maskpred")
                    nc.vector.tensor_tensor(
                        mp[:], msk_s[:, cj * BL:(cj + 1) * BL], pp[:], op=OP.mult)
                    cur = iopool.tile([D + 1, BL], F32, tag="cur")
                    nc.vector.tensor_tensor(
                        r(cur)[0:D, :], mp[:],
                        xmb_s[:, cj * BL:(cj + 1) * BL], op=OP.add)
                    nc.gpsimd.memset(cur[D:D + 1, :], 1.0)
                    if cj == CHUNK - 1 or j == NP - 1:
                        n = (cj + 1) * BL
                        base = (j - cj) * BL
                        nc.sync.dma_start(pred_d[:, base:base + n], po_s[:, 0:n])

                # ---- layer 0 input projection (closes the z0 groups) ----
                for g in GORDER:
                    cs = slice(H * g, H * (g + 1))
                    nc.tensor.matmul(
                        z0b[g][:], r(cur[:]), r(wx0b_s[:, cs]),
                        start=(t == 0), stop=True)

                # ---- layer 1 recurrent part (fills PE during gates0(t)) ----
                z1b = [zpool.tile([32, H], F32, tag="zg", name=f"z1g{g}")
                       for g in range(4)]
                z1_started = (not zero_b1) or t > 0
                for g in GORDER:
                    cs = slice(H * g, H * (g + 1))
                    if not zero_b1:
                        nc.tensor.matmul(
                            z1b[g][:], r(ones_s[:]), r(b1_s[:, cs]),
                            start=True, stop=False)
                    if t > 0:
                        for k in range(KC):
                            nc.tensor.matmul(
                                z1b[g][:], r(h1T[:, 32 * k:32 * (k + 1)]),
                                r(u1_s[:, 4 * H * k + H * g:4 * H * k + H * (g + 1)]),
                                start=(zero_b1 and k == 0), stop=False)

                h0, c0 = gates(z0b, c0, first=(t == 0))
                h0T = transpose_h(h0)

                # ---- layer 1 input part (per-gate so gates1 starts early) ----
                for g in GORDER:
                    for k in range(KC):
                        nc.tensor.matmul(
                            z1b[g][:], r(h0T[:, 32 * k:32 * (k + 1)]),
                            r(wx1_s[:, 4 * H * k + H * g:4 * H * k + H * (g + 1)]),
                            start=(not z1_started and k == 0), stop=(k == KC - 1))

                h1, c1 = gates(z1b, c1, first=(t == 0))
                h1T = transpose_h(h1)

                if t == t_steps - 1:
                    nc.sync.dma_start(last_d[:], h1[:])
                    lp = prpool.tile([BL, C], F32, tag="prps")
                    nc.tensor.matmul(lp[:], r(ones_s[:]), r(bcls_s[:]),
                                     start=True, stop=False)
                    for k in range(KC):
                        nc.tensor.matmul(
                            lp[:], r(h1T[:, 32 * k:32 * (k + 1)]),
                            r(wcls_s[:, C * k:C * (k + 1)]),
                            start=False, stop=(k == KC - 1))
                    lg = gpool.tile([BL, C], F32, tag="logits")
                    nc.vector.tensor_copy(lg[:], lp[:])
                    nc.sync.dma_start(logit_d[:], lg[:])

    nc.compile()
    return nc


# ---------------------------------------------------------------------------
# Host glue
# ---------------------------------------------------------------------------

_NC_CACHE = {}


def _get_nc(t_steps, zero_b1=True):
    key = (t_steps, zero_b1)
    if key not in _NC_CACHE:
        _NC_CACHE[key] = build(t_steps, zero_b1=zero_b1)
    return _NC_CACHE[key]


def make_in_maps(x, Wx0, U0, b0, Wx1, U1, b1, Wproj, bproj, Wcls, bcls,
                 t_steps=T):
    NP = t_steps - 1
    p = GATE_PERM
    wx0b = np.concatenate([np.asarray(Wx0, np.float32),
                           np.asarray(b0, np.float32)[None, :]], axis=0)[:, p]
    shared = dict(
        Wx0b=np.ascontiguousarray(wx0b),
        U0=np.ascontiguousarray(np.asarray(U0, np.float32)[:, p]),
        Wx1=np.ascontiguousarray(np.asarray(Wx1, np.float32)[:, p]),
        U1=np.ascontiguousarray(np.asarray(U1, np.float32)[:, p]),
        b1=np.ascontiguousarray(np.asarray(b1, np.float32)[None, p]),
        Wproj=np.ascontiguousarray(Wproj, np.float32),
        bprojv=np.ascontiguousarray(np.asarray(bproj, np.float32)[:, None]),
        Wcls=np.ascontiguousarray(Wcls, np.float32),
        bcls=np.ascontiguousarray(np.asarray(bcls, np.float32)[None, :]),
    )
    in_maps = []
    for c in range(NCORES):
        xc = np.asarray(x[BL * c:BL * (c + 1), :t_steps], np.float32)
        x0T = np.ascontiguousarray(xc[:, 0, :].T)
        xt = xc[:, 1:, :]
        mask = (xt == MISSING)
        xmb = np.where(mask, 0.0, xt) + mask * np.asarray(bproj, np.float32)[None, None, :]
        xmbT = np.ascontiguousarray(xmb.transpose(2, 1, 0).reshape(D, NP * BL),
                                    np.float32)
        maskT = np.ascontiguousarray(
            mask.astype(np.float32).transpose(2, 1, 0).reshape(D, NP * BL))
        in_maps.append(dict(shared, x0T=x0T, xmbT=xmbT, maskT=maskT))
    return in_maps


def assemble_outputs(results, t_steps=T):
    NP = t_steps - 1
    preds, logits, lasts = [], [], []
    for res in results:
        p = res["pred_out"].reshape(D, NP, BL).transpose(2, 1, 0)
        preds.append(p)
        logits.append(res["logits_out"])
        lasts.append(res["last_out"])
    prediction = np.concatenate(preds, axis=0).reshape(B * NP, D)
    return (prediction.astype(np.float32),
            np.concatenate(logits, axis=0).astype(np.float32),
            np.concatenate(lasts, axis=0).astype(np.float32))


def kernel(x, Wx0, U0, b0, Wx1, U1, b1, Wproj, bproj, Wcls, bcls):
    nc = _get_nc(T, zero_b1=bool(np.all(np.asarray(b1) == 0.0)))
    in_maps = make_in_maps(x, Wx0, U0, b0, Wx1, U1, b1, Wproj, bproj, Wcls, bcls)
    res = run_bass_kernel_spmd(nc, in_maps, core_ids=list(range(NCORES)))
    return assemble_outputs(res.results)


# revision 39
# speedup vs baseline: 1.1203x; 1.1203x over previous
"""AJ-RNN (2-layer LSTM with missing-value imputation) on 8 TRN2 NeuronCores.

Sharding: data-parallel over batch (B=256 -> 32 rows/core), weights replicated,
no collectives. Per core, the T=256 sequential scan runs fully SBUF-resident.

Key hardware facts this design is built around (probed on this toolchain):
  - fp32r matmuls run at 1 cycle/row for N>=256 (4x faster than fp32) but
    REQUIRE dst partition 0 -> no column-tiling; z stays flat [32, 4H].
  - fp32r operands must be produced by an instruction that rounds to fp32r
    (DVE/ACT op with fp32r out dtype); raw DMA data doesn't qualify.
  - tensor_tensor INPUTS must share a start partition; outputs and ACT are
    free. With flat z everything lives at base partition 0 anyway.
  - bf16 anywhere in the recurrence blows the 2e-2 error budget; fp32r
    matmuls + fp32 elementwise measure ~3e-5 end-to-end.

Per-step layout (B_l = 32 batch rows/core):
  - Gate columns host-permuted to (i, f, o, g) so one sigmoid covers
    z[:, 0:1536] and one tanh covers z[:, 1536:2048].
  - b0 folded into the x-projection as a K=33 matmul (cur_aug has a ones row).
  - States kept transposed (hT [128, 4*32]) feeding matmuls as lhsT;
    h -> hT via 4 PE transposes + one fp32r-rounding copy.
  - Imputation: cur^T = xmb^T[t] + mask^T[t] * predraw^T with
    xmb = where(miss,0,x) + miss*bproj precomputed on host;
    predraw^T = Wproj^T @ h1^T computed on-chip (= the `prediction` output).
"""

import numpy as np

import concourse.bass as bass
import concourse.mybir as mybir
from concourse import bacc, masks, tile
from concourse.bass_utils import run_bass_kernel_spmd

F32 = mybir.dt.float32
F32R = mybir.dt.float32r
AF = mybir.ActivationFunctionType
OP = mybir.AluOpType

B, T, D, H, C = 256, 256, 32, 512, 10
NCORES = 8
BL = B // NCORES  # 32
MISSING = 128.0
CHUNK = 16  # io staging chunk, in steps
KC = H // 128  # 4 K-chunks

# host-side gate-column permutation: (i, f, o, g)
GATE_PERM = np.concatenate([
    np.arange(0, H), np.arange(H, 2 * H),
    np.arange(3 * H, 4 * H), np.arange(2 * H, 3 * H)])
# per-gate emission order (f, g, i, o): f's bank finishes first so its
# sigmoid (and the f*c multiply) start while other gates still stream
GORDER = (1, 3, 0, 2)


def r(ap):
    return ap.bitcast(F32R)


def build(t_steps=T, zero_b1=True):
    nc = bacc.Bacc("TRN2", target_bir_lowering=False, debug=False)

    NP = t_steps - 1

    x0T_d = nc.dram_tensor("x0T", [D, BL], F32, kind="ExternalInput")
    xmb_d = nc.dram_tensor("xmbT", [D, NP * BL], F32, kind="ExternalInput")
    msk_d = nc.dram_tensor("maskT", [D, NP * BL], F32, kind="ExternalInput")
    wx0b_d = nc.dram_tensor("Wx0b", [D + 1, 4 * H], F32, kind="ExternalInput")
    u0_d = nc.dram_tensor("U0", [H, 4 * H], F32, kind="ExternalInput")
    wx1_d = nc.dram_tensor("Wx1", [H, 4 * H], F32, kind="ExternalInput")
    u1_d = nc.dram_tensor("U1", [H, 4 * H], F32, kind="ExternalInput")
    b1_d = nc.dram_tensor("b1", [1, 4 * H], F32, kind="ExternalInput")
    wproj_d = nc.dram_tensor("Wproj", [H, D], F32, kind="ExternalInput")
    bprojv_d = nc.dram_tensor("bprojv", [D, 1], F32, kind="ExternalInput")
    wcls_d = nc.dram_tensor("Wcls", [H, C], F32, kind="ExternalInput")
    bcls_d = nc.dram_tensor("bcls", [1, C], F32, kind="ExternalInput")

    pred_d = nc.dram_tensor("pred_out", [D, NP * BL], F32, kind="ExternalOutput")
    last_d = nc.dram_tensor("last_out", [BL, H], F32, kind="ExternalOutput")
    logit_d = nc.dram_tensor("logits_out", [BL, C], F32, kind="ExternalOutput")

    with tile.TileContext(nc) as tc:
        with (
            tc.tile_pool(name="weights", bufs=1) as wpool,
            tc.tile_pool(name="io", bufs=3) as iopool,
            tc.tile_pool(name="state_h", bufs=4) as hpool,
            tc.tile_pool(name="state_c", bufs=3) as cpool,
            tc.tile_pool(name="gates", bufs=2) as gpool,
            tc.tile_pool(name="z0ps", bufs=4, space="PSUM") as z0pool,
            tc.tile_pool(name="z1ps", bufs=3, space="PSUM") as z1pool,
            tc.tile_pool(name="prps", bufs=1, space="PSUM") as prpool,
        ):
            wx0b_s = wpool.tile([D + 1, 4 * H], F32, tag="wx0b")
            u0_s = wpool.tile([128, KC * 4 * H], F32, tag="u0")
            wx1_s = wpool.tile([128, KC * 4 * H], F32, tag="wx1")
            u1_s = wpool.tile([128, KC * 4 * H], F32, tag="u1")
            wproj_s = wpool.tile([128, KC * D], F32, tag="wproj")
            wcls_s = wpool.tile([128, KC * C], F32, tag="wcls")
            b1_s = wpool.tile([1, 4 * H], F32, tag="b1")
            bprojv_s = wpool.tile([D, 1], F32, tag="bprojv")
            bcls_s = wpool.tile([1, C], F32, tag="bcls")
            ones_s = wpool.tile([1, BL], F32, tag="ones")
            x0T_s = wpool.tile([D + 1, BL], F32, tag="x0T")

            # fp32r operands must come from a rounding instruction:
            # DMA -> staging -> rounding copy -> resident tile.
            with tc.tile_pool(name="wstage", bufs=2) as wstage:
                def load_rounded(dst_slice, src_ap, shape):
                    if shape[1] > 2 * H:
                        half = shape[1] // 2
                        load_rounded(dst_slice[:, 0:half], src_ap[:, 0:half],
                                     (shape[0], half))
                        load_rounded(dst_slice[:, half:], src_ap[:, half:],
                                     (shape[0], shape[1] - half))
                        return
                    st = wstage.tile([128, 2 * H], F32, tag="wst")
                    nc.sync.dma_start(st[0:shape[0], 0:shape[1]], src_ap)
                    nc.vector.tensor_copy(
                        dst_slice.bitcast(F32R), st[0:shape[0], 0:shape[1]])

                load_rounded(wx0b_s[:], wx0b_d[:], (D + 1, 4 * H))
                for k in range(KC):
                    sl = slice(128 * k, 128 * (k + 1))
                    load_rounded(u0_s[:, 4 * H * k:4 * H * (k + 1)], u0_d[sl, :], (128, 4 * H))
                    load_rounded(wx1_s[:, 4 * H * k:4 * H * (k + 1)], wx1_d[sl, :], (128, 4 * H))
                    load_rounded(u1_s[:, 4 * H * k:4 * H * (k + 1)], u1_d[sl, :], (128, 4 * H))
                    load_rounded(wproj_s[:, D * k:D * (k + 1)], wproj_d[sl, :], (128, D))
                    load_rounded(wcls_s[:, C * k:C * (k + 1)], wcls_d[sl, :], (128, C))
                load_rounded(b1_s[:], b1_d[:], (1, 4 * H))
                load_rounded(bcls_s[:], bcls_d[:], (1, C))
                load_rounded(x0T_s[0:D, :], x0T_d[:], (D, BL))
            nc.sync.dma_start(bprojv_s[:], bprojv_d[:])
            nc.vector.memset(ones_s[:], 1.0)
            nc.vector.memset(x0T_s[D:D + 1, :], 1.0)

            def hT_chunks(hT):
                return [hT[:, 32 * k:32 * (k + 1)] for k in range(KC)]

            def gates(zb, c_prev, first):
                """zb: 4 PSUM bank tiles in gate order (i, f, o, g).
                ACT/DVE ops emitted in dependency-earliest order (f, g, i, o)
                so they start as soon as each gate's bank finishes."""
                sgf = gpool.tile([32, H], F32, tag="sgf")
                tg = gpool.tile([32, H], F32, tag="tg")
                sgi = gpool.tile([32, H], F32, tag="sgi")
                sgo = gpool.tile([32, H], F32, tag="sgo")
                nc.scalar.activation(sgf[:], zb[1][:], AF.Sigmoid)
                nc.scalar.activation(tg[:], zb[3][:], AF.Tanh)
                nc.scalar.activation(sgi[:], zb[0][:], AF.Sigmoid)
                nc.scalar.activation(sgo[:], zb[2][:], AF.Sigmoid)
                c_new = cpool.tile([32, H], F32, tag="c")
                if first:
                    nc.vector.tensor_tensor(c_new[:], sgi[:], tg[:], op=OP.mult)
                else:
                    fc = gpool.tile([32, H], F32, tag="fc")
                    nc.vector.tensor_tensor(fc[:], sgf[:], c_prev[:], op=OP.mult)
                    m2 = gpool.tile([32, H], F32, tag="m2")
                    nc.vector.tensor_tensor(m2[:], sgi[:], tg[:], op=OP.mult)
                    nc.vector.tensor_tensor(c_new[:], fc[:], m2[:], op=OP.add)
                tc_t = gpool.tile([32, H], F32, tag="tc")
                nc.scalar.activation(tc_t[:], c_new[:], AF.Tanh)
                h = gpool.tile([32, H], F32, tag="h")
                nc.vector.tensor_tensor(h[:], sgo[:], tc_t[:], op=OP.mult)
                return h, c_new

            def transpose_h(h):
                """h [32,512] -> hT [128, 4*32] without touching the PE:
                DVE 32x32 block-transpose, then 16 [32,32] gather copies
                (which also do the fp32r rounding), k-major so the first
                K-chunk of hT is consumable early."""
                tr = gpool.tile([32, H], F32, tag="htr")
                nc.vector.transpose(tr[:], h[:])
                hT = hpool.tile([128, KC * 32], F32, tag="hT")
                for k in range(KC):
                    for c in range(4):
                        dst = r(hT)[32 * c:32 * (c + 1), 32 * k:32 * (k + 1)]
                        src = tr[:, 32 * (4 * k + c):32 * (4 * k + c) + 32]
                        if k < 2:
                            nc.vector.tensor_copy(dst, src)
                        else:
                            nc.scalar.copy(dst, src)
                return hT

            h0T = h1T = c0 = c1 = None
            xmb_s = msk_s = po_s = None

            for t in range(t_steps):
                j = t - 1  # imputation/prediction row index
                cj = j % CHUNK
                if t >= 1 and cj == 0:
                    n = min(CHUNK, NP - j) * BL
                    xmb_s = iopool.tile([D, CHUNK * BL], F32, tag="xmb")
                    msk_s = iopool.tile([D, CHUNK * BL], F32, tag="msk")
                    po_s = iopool.tile([D, CHUNK * BL], F32, tag="po")
                    nc.sync.dma_start(xmb_s[:, 0:n], xmb_d[:, j * BL:j * BL + n])
                    nc.sync.dma_start(msk_s[:, 0:n], msk_d[:, j * BL:j * BL + n])

                # ---- layer-0 recurrent matmuls first: they fill the PE
                # while gates1(t-1) is still on ACT/DVE ----
                z0b = [z0pool.tile([32, H], F32, tag="z0g", name=f"z0g{g}")
                       for g in range(4)]
                if t > 0:
                    for g in GORDER:
                        cs = slice(H * g, H * (g + 1))
                        for k in range(KC):
                            nc.tensor.matmul(
                                z0b[g][:], r(h0T[:, 32 * k:32 * (k + 1)]),
                                r(u0_s[:, 4 * H * k + H * g:4 * H * k + H * (g + 1)]),
                                start=(k == 0), stop=False)

                if t == 0:
                    cur = x0T_s
                else:
                    # predraw^T = Wproj^T @ h1T  [D, BL]
                    pp = prpool.tile([D, BL], F32, tag="prps")
                    for k in range(KC):
                        nc.tensor.matmul(
                            pp[:], r(wproj_s[:, D * k:D * (k + 1)]),
                            r(h1T[:, 32 * k:32 * (k + 1)]),
                            start=(k == 0), stop=(k == KC - 1),
                        )
                    nc.vector.tensor_scalar_add(
                        po_s[:, cj * BL:(cj + 1) * BL], pp[:], bprojv_s[:])
                    # cur^T = xmb^T + mask^T * predraw^T
                    mp = iopool.tile([D, BL], F32, tag="maskpred")
                    nc.vector.tensor_tensor(
                        mp[:], msk_s[:, cj * BL:(cj + 1) * BL], pp[:], op=OP.mult)
                    cur = iopool.tile([D + 1, BL], F32, tag="cur")
                    nc.vector.tensor_tensor(
                        r(cur)[0:D, :], mp[:],
                        xmb_s[:, cj * BL:(cj + 1) * BL], op=OP.add)
                    nc.gpsimd.memset(cur[D:D + 1, :], 1.0)
                    if cj == CHUNK - 1 or j == NP - 1:
                        n = (cj + 1) * BL
                        base = (j - cj) * BL
                        nc.sync.dma_start(pred_d[:, base:base + n], po_s[:, 0:n])

                # ---- layer 0 input projection (closes the z0 groups) ----
                for g in GORDER:
                    cs = slice(H * g, H * (g + 1))
                    nc.tensor.matmul(
                        z0b[g][:], r(cur[:]), r(wx0b_s[:, cs]),
                        start=(t == 0), stop=True)

                # ---- layer 1 recurrent part (fills PE during gates0(t)) ----
                z1b = [z1pool.tile([32, H], F32, tag="z1g", name=f"z1g{g}")
                       for g in range(4)]
                z1_started = (not zero_b1) or t > 0
                for g in GORDER:
                    cs = slice(H * g, H * (g + 1))
                    if not zero_b1:
                        nc.tensor.matmul(
                            z1b[g][:], r(ones_s[:]), r(b1_s[:, cs]),
                            start=True, stop=False)
                    if t > 0:
                        for k in range(KC):
                            nc.tensor.matmul(
                                z1b[g][:], r(h1T[:, 32 * k:32 * (k + 1)]),
                                r(u1_s[:, 4 * H * k + H * g:4 * H * k + H * (g + 1)]),
                                start=(zero_b1 and k == 0), stop=False)

                h0, c0 = gates(z0b, c0, first=(t == 0))
                h0T = transpose_h(h0)

                # ---- layer 1 input part (per-gate so gates1 starts early) ----
                for g in GORDER:
                    for k in range(KC):
                        nc.tensor.matmul(
                            z1b[g][:], r(h0T[:, 32 * k:32 * (k + 1)]),
                            r(wx1_s[:, 4 * H * k + H * g:4 * H * k + H * (g + 1)]),
                            start=(not z1_started and k == 0), stop=(k == KC - 1))

                h1, c1 = gates(z1b, c1, first=(t == 0))
                h1T = transpose_h(h1)

                if t == t_steps - 1:
                    nc.sync.dma_start(last_d[:], h1[:])
                    lp = prpool.tile([BL, C], F32, tag="prps")
                    nc.tensor.matmul(lp[:], r(ones_s[:]), r(bcls_s[:]),
                                     start=True, stop=False)
                    for k in range(KC):
                        nc.tensor.matmul(
                            lp[:], r(h1T[:, 32 * k:32 * (k + 1)]),
                            r(wcls_s[:, C * k:C * (k + 1)]),
                            start=False, stop=(k == KC - 1))
                    lg = gpool.tile([BL, C], F32, tag="logits")
                    nc.vector.tensor_copy(lg[:], lp[:])
                    nc.sync.dma_start(logit_d[:], lg[:])

    nc.compile()
    return nc


# ---------------------------------------------------------------------------
# Host glue
# ---------------------------------------------------------------------------

_NC_CACHE = {}


def _get_nc(t_steps, zero_b1=True):
    key = (t_steps, zero_b1)
    if key not in _NC_CACHE:
        _NC_CACHE[key] = build(t_steps, zero_b1=zero_b1)
    return _NC_CACHE[key]


def make_in_maps(x, Wx0, U0, b0, Wx1, U1, b1, Wproj, bproj, Wcls, bcls,
                 t_steps=T):
    NP = t_steps - 1
    p = GATE_PERM
    wx0b = np.concatenate([np.asarray(Wx0, np.float32),
                           np.asarray(b0, np.float32)[None, :]], axis=0)[:, p]
    shared = dict(
        Wx0b=np.ascontiguousarray(wx0b),
        U0=np.ascontiguousarray(np.asarray(U0, np.float32)[:, p]),
        Wx1=np.ascontiguousarray(np.asarray(Wx1, np.float32)[:, p]),
        U1=np.ascontiguousarray(np.asarray(U1, np.float32)[:, p]),
        b1=np.ascontiguousarray(np.asarray(b1, np.float32)[None, p]),
        Wproj=np.ascontiguousarray(Wproj, np.float32),
        bprojv=np.ascontiguousarray(np.asarray(bproj, np.float32)[:, None]),
        Wcls=np.ascontiguousarray(Wcls, np.float32),
        bcls=np.ascontiguousarray(np.asarray(bcls, np.float32)[None, :]),
    )
    in_maps = []
    for c in range(NCORES):
        xc = np.asarray(x[BL * c:BL * (c + 1), :t_steps], np.float32)
        x0T = np.ascontiguousarray(xc[:, 0, :].T)
        xt = xc[:, 1:, :]
        mask = (xt == MISSING)
        xmb = np.where(mask, 0.0, xt) + mask * np.asarray(bproj, np.float32)[None, None, :]
        xmbT = np.ascontiguousarray(xmb.transpose(2, 1, 0).reshape(D, NP * BL),
                                    np.float32)
        maskT = np.ascontiguousarray(
            mask.astype(np.float32).transpose(2, 1, 0).reshape(D, NP * BL))
        in_maps.append(dict(shared, x0T=x0T, xmbT=xmbT, maskT=maskT))
    return in_maps


def assemble_outputs(results, t_steps=T):
    NP = t_steps - 1
    preds, logits, lasts = [], [], []
    for res in results:
        p = res["pred_out"].reshape(D, NP, BL).transpose(2, 1, 0)
        preds.append(p)
        logits.append(res["logits_out"])
        lasts.append(res["last_out"])
    prediction = np.concatenate(preds, axis=0).reshape(B * NP, D)
    return (prediction.astype(np.float32),
            np.concatenate(logits, axis=0).astype(np.float32),
            np.concatenate(lasts, axis=0).astype(np.float32))


def kernel(x, Wx0, U0, b0, Wx1, U1, b1, Wproj, bproj, Wcls, bcls):
    nc = _get_nc(T, zero_b1=bool(np.all(np.asarray(b1) == 0.0)))
    in_maps = make_in_maps(x, Wx0, U0, b0, Wx1, U1, b1, Wproj, bproj, Wcls, bcls)
    res = run_bass_kernel_spmd(nc, in_maps, core_ids=list(range(NCORES)))
    return assemble_outputs(res.results)
